# revision 19
# baseline (speedup 1.0000x reference)
"""LocalDensityEncoder on 8 Trainium2 NeuronCores.

Sharding: node rows are split across 8 cores (2048 rows each). Each core:
  - density: computes its 2048 x 16384 block of the pairwise-distance
    indicator via a K=4 augmented matmul producing z=(R^2-d2)/2 directly in
    PSUM, then counts in-radius neighbours with VE is_ge(+accum) and ACT
    Sign(+accum) split across chunks.
  - edges: host counting-sorts edges into per-row-tile buckets (tile-major
    padded layout); core gathers x[col] via indirect DMA and segment-sums via
    a one-hot matmul S.T @ [X|1] into PSUM (nbr_sum and degree).
  - fvar, global maxima via in-kernel AllReduce(max), and the 2-layer MLP in
    transposed layout.
Host does only index bucketing/padding + slicing; all arithmetic on device.
"""

import numpy as np

N = 16384
E = 524288
H = 256
NCORES = 8
ROWS = N // NCORES            # 2048 rows per core
P = 128
NTILE = ROWS // P             # 16 row tiles per core
JCH = 512                     # density j-chunk width
NJC = N // JCH                # 32 j chunks
CAP_CH = 36                   # edge chunks (of 128 edges) per row tile
R2 = 2500.0
EPS = 1e-8
NSIGN = 32                    # of NJC chunks handled by ACT Sign (rest VE is_ge)
OOB = 10 * N                  # sentinel col index for padded edges (skipped)

_CACHE = {}


def _build():
    import concourse.bacc as bacc
    import concourse.bass as bass
    import concourse.mybir as mybir
    import concourse.tile as tile
    from concourse.masks import make_identity

    dt = mybir.dt
    f32 = dt.float32
    bf16 = dt.bfloat16
    i32 = dt.int32
    A = mybir.AluOpType
    AF = mybir.ActivationFunctionType

    nc = bacc.Bacc("TRN2", target_bir_lowering=False, debug=False,
                   enable_asserts=False, num_devices=NCORES)

    x_d = nc.dram_tensor("x", [N, H], f32, kind="ExternalInput")
    coords_d = nc.dram_tensor("coords", [N, 2], f32, kind="ExternalInput")
    cown_d = nc.dram_tensor("cown", [ROWS, 2], f32, kind="ExternalInput")
    xrows_d = nc.dram_tensor("xrows", [ROWS, H], f32, kind="ExternalInput")
    cols_d = nc.dram_tensor("cols", [NTILE, P, CAP_CH], i32, kind="ExternalInput")
    rrel_d = nc.dram_tensor("rrel", [NTILE, P, CAP_CH], f32, kind="ExternalInput")
    w1_d = nc.dram_tensor("w1", [3, H // 2], f32, kind="ExternalInput")
    b1_d = nc.dram_tensor("b1", [H // 2, 1], f32, kind="ExternalInput")
    w2_d = nc.dram_tensor("w2", [H // 2, H], f32, kind="ExternalInput")
    b2_d = nc.dram_tensor("b2", [1, H], f32, kind="ExternalInput")
    out_d = nc.dram_tensor("out", [ROWS, H], f32, kind="ExternalOutput")
    import os
    dbg = os.environ.get("KERNEL_DEBUG", "0") == "1"
    if dbg:
        dbg_d = nc.dram_tensor("dbg", [P, 3 * NTILE], f32, kind="ExternalOutput")
        dbg2_d = nc.dram_tensor("dbg2", [P, NTILE * H], f32, kind="ExternalOutput")
        dbg3_d = nc.dram_tensor("dbg3", [4, N], f32, kind="ExternalOutput")
        dbg4_d = nc.dram_tensor("dbg4", [4, ROWS], f32, kind="ExternalOutput")
        dbg5_d = nc.dram_tensor("dbg5", [P, H + P], f32, kind="ExternalOutput")
        dbg6_d = nc.dram_tensor("dbg6", [P, NJC], f32, kind="ExternalOutput")

    sign_chunk = [False] * NJC
    for k in range(NSIGN):                      # spread ACT chunks evenly
        sign_chunk[(k * NJC) // NSIGN] = True

    with tile.TileContext(nc) as tc:
        with (
            tc.tile_pool(name="const", bufs=1) as cpool,
            tc.tile_pool(name="work", bufs=3) as wpool,
            tc.tile_pool(name="gath", bufs=8) as gpool,
            tc.tile_pool(name="zps", bufs=2, space="PSUM") as zpool,
            tc.tile_pool(name="nps", bufs=2, space="PSUM") as npool,
            tc.tile_pool(name="mps", bufs=1, space="PSUM") as mpool,
            tc.tile_pool(name="dram", bufs=1, space="DRAM") as dpool,
        ):
            # ---------------- constants / prep ----------------
            ident = cpool.tile([P, P], f32)
            make_identity(nc, ident[:])
            iota_f = cpool.tile([P, P], f32)
            iota_i = cpool.tile([P, P], i32)
            nc.gpsimd.iota(iota_i[:], pattern=[[1, P]], base=0, channel_multiplier=0)
            nc.vector.tensor_copy(iota_f[:], iota_i[:])
            ones_b = cpool.tile([P, 1], bf16)
            nc.vector.memset(ones_b[:], 1.0)
            ones_row = cpool.tile([1, P], f32)
            nc.vector.memset(ones_row[:], 1.0)

            w1s = cpool.tile([3, H // 2], f32)
            nc.sync.dma_start(w1s[:], w1_d[:])
            b1c = cpool.tile([H // 2, 1], f32)
            nc.sync.dma_start(b1c[:], b1_d[:])
            w2s = cpool.tile([H // 2, H], f32)
            nc.sync.dma_start(w2s[:], w2_d[:])
            b2r = cpool.tile([1, H], f32)
            nc.sync.dma_start(b2r[:], b2_d[:])
            # broadcast b2 down partitions via ones x b2 matmul
            ps_b2 = mpool.tile([P, H], f32, space="PSUM", tag="m")
            nc.tensor.matmul(out=ps_b2[:], lhsT=ones_row[:], rhs=b2r[:],
                             start=True, stop=True)
            b2bc = cpool.tile([P, H], f32)
            nc.vector.tensor_copy(b2bc[:], ps_b2[:])

            # ---------------- augmented coord arrays ----------------
            # AUGJ rows (moving): x_j, y_j, -sq_j/2, 1
            # AUGI rows (stationary): x_i, y_i, 1, (R2-sq_i)/2
            AUGJ = cpool.tile([4, N], f32)
            AUGI = cpool.tile([4, ROWS], f32)
            ones_big = cpool.tile([P, P], f32)
            nc.vector.memset(ones_big[:], 1.0)
            nc.sync.dma_start(AUGJ[3:4, :].rearrange("r (p t) -> r p t", p=P),
                              ones_big[:])
            nc.sync.dma_start(AUGI[2:3, :].rearrange("r (p t) -> r p t", p=P),
                              ones_big[:, 0:NTILE])

            # all-node coords in (p,t) layout: [128, 128*2], node = p*128+t
            cpt = wpool.tile([P, P * 2], f32, tag="prep")
            nc.sync.dma_start(cpt[:], coords_d.ap().rearrange("(p t) c -> p (t c)", p=P))
            cxy = cpt[:].rearrange("p (t c) -> p t c", c=2)
            xpt = wpool.tile([P, P], f32, tag="prep")
            ypt = wpool.tile([P, P], f32, tag="prep")
            nc.vector.tensor_copy(xpt[:], cxy[:, :, 0])
            nc.vector.tensor_copy(ypt[:], cxy[:, :, 1])
            sqp = wpool.tile([P, P], f32, tag="prep")
            nc.vector.tensor_tensor(out=sqp[:], in0=xpt[:], in1=xpt[:], op=A.mult)
            typ = wpool.tile([P, P], f32, tag="prep")
            nc.vector.tensor_tensor(out=typ[:], in0=ypt[:], in1=ypt[:], op=A.mult)
            nc.vector.tensor_tensor(out=sqp[:], in0=sqp[:], in1=typ[:], op=A.add)
            c3p = wpool.tile([P, P], f32, tag="prep")
            nc.vector.tensor_scalar(out=c3p[:], in0=sqp[:], scalar1=-0.5,
                                    scalar2=None, op0=A.mult)
            # flatten (partition-major) into AUGJ rows via SBUF->SBUF DMA
            nc.sync.dma_start(AUGJ[0:1, :].rearrange("r (p t) -> r p t", p=P), xpt[:])
            nc.sync.dma_start(AUGJ[1:2, :].rearrange("r (p t) -> r p t", p=P), ypt[:])
            nc.sync.dma_start(AUGJ[2:3, :].rearrange("r (p t) -> r p t", p=P), c3p[:])

            # own-row coords, (p,t) layout with t=NTILE
            copt = wpool.tile([P, NTILE * 2], f32, tag="prep2")
            nc.sync.dma_start(copt[:], cown_d.ap().rearrange("(p t) c -> p (t c)", p=P))
            coxy = copt[:].rearrange("p (t c) -> p t c", c=2)
            xo = wpool.tile([P, NTILE], f32, tag="prep2")
            yo = wpool.tile([P, NTILE], f32, tag="prep2")
            nc.vector.tensor_copy(xo[:], coxy[:, :, 0])
            nc.vector.tensor_copy(yo[:], coxy[:, :, 1])
            sqo = wpool.tile([P, NTILE], f32, tag="prep2")
            nc.vector.tensor_tensor(out=sqo[:], in0=xo[:], in1=xo[:], op=A.mult)
            tyo = wpool.tile([P, NTILE], f32, tag="prep2")
            nc.vector.tensor_tensor(out=tyo[:], in0=yo[:], in1=yo[:], op=A.mult)
            nc.vector.tensor_tensor(out=sqo[:], in0=sqo[:], in1=tyo[:], op=A.add)
            c4o = wpool.tile([P, NTILE], f32, tag="prep2")
            nc.vector.tensor_scalar(out=c4o[:], in0=sqo[:], scalar1=-0.5,
                                    scalar2=R2 / 2.0, op0=A.mult, op1=A.add)
            nc.sync.dma_start(AUGI[0:1, :].rearrange("r (p t) -> r p t", p=P), xo[:])
            nc.sync.dma_start(AUGI[1:2, :].rearrange("r (p t) -> r p t", p=P), yo[:])
            nc.sync.dma_start(AUGI[3:4, :].rearrange("r (p t) -> r p t", p=P), c4o[:])

            # NOTE: AUGI/AUGJ free index within row r is node (p*128+t) given the
            # (p,t) source layout; our DMA pairs source AP order (p-major) with a
            # linear destination, so AUGJ[:, j] == node j. Good.

            # stat columns per row tile
            degc = cpool.tile([P, NTILE], f32)
            drawc = cpool.tile([P, NTILE], f32)
            fvc = cpool.tile([P, NTILE], f32)

            # ---------------- main per-tile loops ----------------
            for it in range(NTILE):
                lhs_i = AUGI[:, it * P:(it + 1) * P]

                # --- density ---
                NVE = NJC - NSIGN
                colcnt = wpool.tile([P, NVE], f32, tag="colcnt") if NVE > 0 else None
                colsgn = wpool.tile([P, NSIGN], f32, tag="colsgn")
                nve = nsg = 0
                for jc in range(NJC):
                    z = zpool.tile([P, JCH], f32, space="PSUM", tag="z")
                    nc.tensor.matmul(out=z[:], lhsT=lhs_i,
                                     rhs=AUGJ[:, jc * JCH:(jc + 1) * JCH],
                                     start=True, stop=True)
                    scr = wpool.tile([P, JCH], bf16, tag="scr")
                    if sign_chunk[jc]:
                        nc.scalar.activation(out=scr[:], in_=z[:], func=AF.Sign,
                                             accum_out=colsgn[:, nsg:nsg + 1])
                        nsg += 1
                    else:
                        nc.vector.tensor_scalar(out=scr[:], in0=z[:], scalar1=0.0,
                                                scalar2=1.0, op0=A.is_ge,
                                                op1=A.mult,
                                                accum_out=colcnt[:, nve:nve + 1])
                        nve += 1
                sgnB = wpool.tile([P, 1], f32, tag="sB")
                if dbg and it == 0:
                    nc.sync.dma_start(dbg6_d[:, 0:NSIGN], colsgn[:])
                nc.vector.tensor_reduce(out=sgnB[:], in_=colsgn[:],
                                        axis=mybir.AxisListType.X, op=A.add)
                # draw = cntA + sgnB/2 + (NSIGN*JCH/2 - 1)
                nc.vector.tensor_scalar(out=sgnB[:], in0=sgnB[:], scalar1=0.5,
                                        scalar2=float(NSIGN * JCH // 2 - 1),
                                        op0=A.mult, op1=A.add)
                if NVE > 0:
                    cntA = wpool.tile([P, 1], f32, tag="cA")
                    nc.vector.tensor_reduce(out=cntA[:], in_=colcnt[:],
                                            axis=mybir.AxisListType.X, op=A.add)
                    nc.vector.tensor_tensor(out=drawc[:, it:it + 1], in0=cntA[:],
                                            in1=sgnB[:], op=A.add)
                else:
                    nc.vector.tensor_copy(drawc[:, it:it + 1], sgnB[:])

                # --- edges: gather + segment matmul ---
                rrt = wpool.tile([P, CAP_CH], f32, tag="rrt")
                nc.sync.dma_start(rrt[:], rrel_d[it])
                clt = wpool.tile([P, CAP_CH], i32, tag="clt")
                nc.sync.dma_start(clt[:], cols_d[it])
                ps_nb = npool.tile([P, H], f32, space="PSUM", tag="nb")
                ps_dg = npool.tile([P, 8], f32, space="PSUM", tag="dg")
                for c in range(CAP_CH):
                    import concourse.bass as _b
                    Xg = gpool.tile([P, H], bf16, tag="xg")
                    nc.gpsimd.indirect_dma_start(
                        out=Xg[:], out_offset=None, in_=x_d[:],
                        in_offset=_b.IndirectOffsetOnAxis(ap=clt[:, c:c + 1], axis=0),
                        bounds_check=N - 1, oob_is_err=False)
                    S = gpool.tile([P, P], bf16, tag="S")
                    nc.vector.tensor_scalar(out=S[:], in0=iota_f[:],
                                            scalar1=rrt[:, c:c + 1], scalar2=None,
                                            op0=A.is_equal)
                    nc.tensor.matmul(out=ps_nb[:], lhsT=S[:], rhs=Xg[:],
                                     start=(c == 0), stop=(c == CAP_CH - 1))
                    nc.tensor.matmul(out=ps_dg[:, 0:1], lhsT=S[:], rhs=ones_b[:],
                                     start=(c == 0), stop=(c == CAP_CH - 1))
                    if dbg and it == 0 and c == 0:
                        xgf5 = wpool.tile([P, H], f32, tag="xgf5")
                        nc.vector.tensor_copy(xgf5[:], Xg[:])
                        nc.sync.dma_start(dbg5_d[:, 0:H], xgf5[:])
                        sf5 = wpool.tile([P, P], f32, tag="sf5")
                        nc.vector.tensor_copy(sf5[:], S[:])
                        nc.sync.dma_start(dbg5_d[:, H:H + P], sf5[:])

                # --- fvar ---
                nc.vector.tensor_copy(degc[:, it:it + 1], ps_dg[:, 0:1])
                cnt1 = wpool.tile([P, 1], f32, tag="cnt1")
                nc.vector.tensor_scalar(out=cnt1[:], in0=ps_dg[:, 0:1],
                                        scalar1=1.0, scalar2=None, op0=A.max)
                rcp = wpool.tile([P, 1], f32, tag="rcp")
                nc.vector.reciprocal(rcp[:], cnt1[:])
                nm = wpool.tile([P, H], f32, tag="nm")
                nc.vector.tensor_scalar(out=nm[:], in0=ps_nb[:],
                                        scalar1=rcp[:, 0:1], scalar2=None, op0=A.mult)
                xr = wpool.tile([P, H], f32, tag="xr")
                nc.sync.dma_start(xr[:], xrows_d[it * P:(it + 1) * P, :])
                dif = wpool.tile([P, H], f32, tag="dif")
                nc.vector.tensor_tensor(out=dif[:], in0=xr[:], in1=nm[:], op=A.subtract)
                if dbg:
                    nc.sync.dma_start(dbg2_d[:, it * H:(it + 1) * H], nm[:])
                scrd = wpool.tile([P, H], bf16, tag="scrd")
                fv2 = wpool.tile([P, 1], f32, tag="fv2")
                nc.scalar.activation(out=scrd[:], in_=dif[:], func=AF.Square,
                                     accum_out=fv2[:, 0:1])
                nc.scalar.activation(out=fvc[:, it:it + 1], in_=fv2[:], func=AF.Sqrt)

            if dbg:
                nc.sync.dma_start(dbg3_d[:], AUGJ[:])
                nc.sync.dma_start(dbg4_d[:], AUGI[:])
                nc.sync.dma_start(dbg_d[:, 0:NTILE], degc[:])
                nc.sync.dma_start(dbg_d[:, NTILE:2 * NTILE], drawc[:])
                nc.sync.dma_start(dbg_d[:, 2 * NTILE:3 * NTILE], fvc[:])

            # ---------------- global maxima ----------------
            locm = cpool.tile([P, 4], f32)
            nc.vector.tensor_reduce(out=locm[:, 0:1], in_=degc[:],
                                    axis=mybir.AxisListType.X, op=A.max)
            nc.vector.tensor_reduce(out=locm[:, 1:2], in_=drawc[:],
                                    axis=mybir.AxisListType.X, op=A.max)
            nc.vector.tensor_reduce(out=locm[:, 2:3], in_=fvc[:],
                                    axis=mybir.AxisListType.X, op=A.max)
            nc.vector.memset(locm[:, 3:4], 0.0)
            ps_t = mpool.tile([4, P], f32, space="PSUM", tag="m")
            nc.tensor.transpose(out=ps_t[:], in_=locm[:], identity=ident[:])
            lmax = cpool.tile([4, 1], f32)
            nc.vector.tensor_reduce(out=lmax[:], in_=ps_t[:],
                                    axis=mybir.AxisListType.X, op=A.max)
            ar_in = dpool.tile([4, 8], f32)
            ar_out = dpool.tile([4, 8], f32)
            lmax8 = cpool.tile([4, 8], f32)
            nc.vector.tensor_copy(lmax8[:], lmax[:].to_broadcast([4, 8]))
            nc.sync.dma_start(ar_in[:], lmax8[:])
            nc.gpsimd.collective_compute(
                "AllReduce", A.max, replica_groups=[list(range(NCORES))],
                ins=[ar_in[:]], outs=[ar_out[:]])
            gmax = cpool.tile([4, 8], f32)
            nc.sync.dma_start(gmax[:], ar_out[:])
            den = cpool.tile([4, 1], f32)
            nc.vector.tensor_scalar(out=den[:], in0=gmax[:, 0:1], scalar1=EPS,
                                    scalar2=None, op0=A.add)
            rec = cpool.tile([4, 1], f32)
            nc.vector.reciprocal(rec[:], den[:])
            # transpose rec -> [1,4] then broadcast down partitions
            ps_r = mpool.tile([4, P], f32, space="PSUM", tag="m")
            nc.tensor.transpose(out=ps_r[0:1, 0:4], in_=rec[:], identity=ident[0:4, 0:4])
            recr = cpool.tile([1, 4], f32)
            nc.vector.tensor_copy(recr[:], ps_r[0:1, 0:4])
            ps_rb = mpool.tile([P, 4], f32, space="PSUM", tag="m")
            nc.tensor.matmul(out=ps_rb[:], lhsT=ones_row[:], rhs=recr[:],
                             start=True, stop=True)
            recbc = cpool.tile([P, 4], f32)
            nc.vector.tensor_copy(recbc[:], ps_rb[:])

            # ---------------- feats + MLP ----------------
            for it in range(NTILE):
                st3 = wpool.tile([P, 4], f32, tag="st3")
                nc.vector.tensor_copy(st3[:, 0:1], degc[:, it:it + 1])
                nc.vector.tensor_copy(st3[:, 1:2], drawc[:, it:it + 1])
                nc.vector.tensor_copy(st3[:, 2:3], fvc[:, it:it + 1])
                nc.vector.memset(st3[:, 3:4], 0.0)
                nc.vector.tensor_tensor(out=st3[:], in0=st3[:], in1=recbc[:], op=A.mult)
                ps_f = mpool.tile([4, P], f32, space="PSUM", tag="m")
                nc.tensor.transpose(out=ps_f[:], in_=st3[:], identity=ident[:])
                fT = wpool.tile([4, P], f32, tag="fT")
                nc.vector.tensor_copy(fT[:], ps_f[:])
                ps_h = mpool.tile([H // 2, P], f32, space="PSUM", tag="m")
                nc.tensor.matmul(out=ps_h[:], lhsT=w1s[:], rhs=fT[0:3, :],
                                 start=True, stop=True)
                hT = wpool.tile([H // 2, P], f32, tag="hT")
                nc.scalar.activation(out=hT[:], in_=ps_h[:], func=AF.Relu,
                                     bias=b1c[:, 0:1], scale=1.0)
                ps_o = mpool.tile([P, H], f32, space="PSUM", tag="m")
                nc.tensor.matmul(out=ps_o[:], lhsT=hT[:], rhs=w2s[:],
                                 start=True, stop=True)
                outs = wpool.tile([P, H], f32, tag="outs")
                nc.vector.tensor_tensor(out=outs[:], in0=ps_o[:], in1=b2bc[:], op=A.add)
                nc.sync.dma_start(out_d[it * P:(it + 1) * P, :], outs[:])

    nc.compile()
    return nc


def _prep_inputs(x, edge_index, spatial_coords, w1, b1, w2, b2):
    row = np.asarray(edge_index[0], np.int64)
    col = np.asarray(edge_index[1], np.int64)
    order = np.argsort(row, kind="stable")
    srow = row[order]
    scol = col[order]
    g = (srow // P).astype(np.int64)
    counts = np.bincount(g, minlength=N // P)
    starts = np.zeros(N // P + 1, np.int64)
    np.cumsum(counts, out=starts[1:])
    cols_all = np.full((NCORES, NTILE, P, CAP_CH), OOB, np.int32)
    rrel_all = np.full((NCORES, NTILE, P, CAP_CH), -1.0, np.float32)
    for gt in range(N // P):
        cnt = counts[gt]
        if cnt > P * CAP_CH:
            raise RuntimeError(f"tile {gt} overflow: {cnt}")
        d, t = divmod(gt, NTILE)
        seg = slice(starts[gt], starts[gt] + cnt)
        idx = np.arange(cnt)
        p = idx % P
        c = idx // P
        cols_all[d, t, p, c] = scol[seg].astype(np.int32)
        rrel_all[d, t, p, c] = (srow[seg] - gt * P).astype(np.float32)

    x = np.ascontiguousarray(x, np.float32)
    coords = np.ascontiguousarray(spatial_coords, np.float32)
    in_maps = []
    for d in range(NCORES):
        in_maps.append({
            "x": x,
            "coords": coords,
            "cown": np.ascontiguousarray(coords[d * ROWS:(d + 1) * ROWS]),
            "xrows": np.ascontiguousarray(x[d * ROWS:(d + 1) * ROWS]),
            "cols": np.ascontiguousarray(cols_all[d]),
            "rrel": np.ascontiguousarray(rrel_all[d]),
            "w1": np.ascontiguousarray(w1, np.float32),
            "b1": np.ascontiguousarray(np.asarray(b1, np.float32).reshape(H // 2, 1)),
            "w2": np.ascontiguousarray(w2, np.float32),
            "b2": np.ascontiguousarray(np.asarray(b2, np.float32).reshape(1, H)),
        })
    return in_maps


def kernel(x, edge_index, spatial_coords, w1, b1, w2, b2):
    from concourse.bass_utils import run_bass_kernel_spmd
    if "nc" not in _CACHE:
        _CACHE["nc"] = _build()
    nc = _CACHE["nc"]
    in_maps = _prep_inputs(x, edge_index, spatial_coords, w1, b1, w2, b2)
    res = run_bass_kernel_spmd(nc, in_maps, core_ids=list(range(NCORES)))
    out = np.concatenate([res.results[d]["out"] for d in range(NCORES)], axis=0)
    return out.astype(np.float32)
